# revision 16
# baseline (speedup 1.0000x reference)
"""GQA forward (b=2, s=2048, H=32 q heads, 8 kv heads, d=64) on 8 TRN2 cores.

Sharding: core k owns query heads 4k..4k+3 and kv head k. GQA group
structure makes attention fully local per core (q heads 4k..4k+3 attend
only to kv head k). x is replicated; W columns are sharded; outputs are
column-concatenated.

Per-core kernel (Tile framework), fp16 datapath / fp32 accumulation:
  - x.T is produced on the HOST (numpy transpose + fp16 cast) and DMA'd
    straight into SBUF — no on-chip transposes of x.
  - W-stationary projections: QKV.T[wcol, s] = W_chunk.T @ xT accumulated
    in fp32 PSUM over 16 k-chunks, 512-wide moving — Q.T/K.T come out
    already in [d, s] layout, no flip transposes. W columns are permuted
    on the host so each head's rows are [evens(32) | odds(32)] (RoPE
    pairs de-interleaved; scores are invariant to a shared d-permutation
    of Q and K).
  - RoPE on DVE in [d, s] layout: partner rows fetched with 32-partition
    cross-quadrant copies, then out = ppt*cos + partner*sin(signed) as
    three [128, 512] tensor_tensor ops fused with the PSUM->SBUF
    eviction (V rows pass through via cos=1/sin=0 table entries).
  - V.T flipped back to [kv, d] via 4 small PE transposes per s-tile.
  - Attention in transposed layout: S.T[kv,q] = K @ Q.T per 128-kv block,
    exp on ACT (scale=1/8 folded in) with fp16 output, causal handled by
    skipping blocks above the diagonal + multiplying the diagonal block
    of P by a 0/1 fp16 mask, ctx.T[65,q] = [V|1].T @ P.T accumulated in
    fp32 PSUM (row 64 = softmax sums).
  - Finalize: 4 PE transposes of ctx.T into one [128, 4x66] PSUM tile
    (shared with the cxt pool), one reciprocal, 4 scalar muls, one DMA
    per (head, s-tile).
"""

import numpy as np
from contextlib import ExitStack

import concourse.bass as bass
import concourse.bacc as bacc
import concourse.mybir as mybir
from concourse import tile
from concourse.bass_utils import run_bass_kernel_spmd

F32 = mybir.dt.float32
F16 = mybir.dt.float16
MUL = mybir.AluOpType.mult
ADD = mybir.AluOpType.add

B = 2
S = 2048
DIN = 2048
D = 64              # head dim
HPC = 4             # query heads per core
NCORES = 8
WCOLS = 4 * D + D + D  # 256 q cols + 64 k + 64 v = 384
ST = 512            # s-tile (rows per outer step)
NST = B * S // ST   # 8 s-tiles
NCH = DIN // 128    # 16 k-chunks
NKV = S // 128      # kv tiles per batch


def build_bass():
    nc = bacc.Bacc(None, target_bir_lowering=False)
    xt_d = nc.declare_dram_parameter("xt", [DIN, B * S], F16, isOutput=False)
    w_d = nc.declare_dram_parameter("w", [DIN, WCOLS], F16, isOutput=False)
    cq_d = nc.declare_dram_parameter("cq", [128, S], F16, isOutput=False)
    sq_d = nc.declare_dram_parameter("sq", [128, S], F16, isOutput=False)
    ck_d = nc.declare_dram_parameter("ck", [128, S], F16, isOutput=False)
    sk_d = nc.declare_dram_parameter("sk", [128, S], F16, isOutput=False)
    mask_d = nc.declare_dram_parameter("mask", [128, 128], F16, isOutput=False)
    idlo_d = nc.declare_dram_parameter("idlo", [128, 64], F16, isOutput=False)
    id32_d = nc.declare_dram_parameter("id32", [128, 128], F32, isOutput=False)
    out_d = nc.declare_dram_parameter("out", [B * S, HPC * D], F32, isOutput=True)

    with ExitStack() as ctx:
        tc = ctx.enter_context(tile.TileContext(nc))
        const = ctx.enter_context(tc.tile_pool(name="const", bufs=1))
        resid = ctx.enter_context(tc.tile_pool(name="resid", bufs=1))
        xt_p = ctx.enter_context(tc.tile_pool(name="xt", bufs=2))
        ro_p = ctx.enter_context(tc.tile_pool(name="ro", bufs=2))
        qt_p = ctx.enter_context(tc.tile_pool(name="qt", bufs=4))
        kvq_p = ctx.enter_context(tc.tile_pool(name="kvq", bufs=2))
        p_p = ctx.enter_context(tc.tile_pool(name="p", bufs=3))
        cx_p = ctx.enter_context(tc.tile_pool(name="cx", bufs=2))
        o_p = ctx.enter_context(tc.tile_pool(name="o", bufs=3))
        rv_p = ctx.enter_context(tc.tile_pool(name="rv", bufs=3))
        tp_ps = ctx.enter_context(tc.tile_pool(name="tp_ps", bufs=2, space="PSUM"))
        pr_ps = ctx.enter_context(tc.tile_pool(name="pr_ps", bufs=2, space="PSUM"))
        sc_ps = ctx.enter_context(tc.tile_pool(name="sc_ps", bufs=2, space="PSUM"))
        cx_ps = ctx.enter_context(tc.tile_pool(name="cx_ps", bufs=2, space="PSUM"))

        # constants on the SCALAR engine's DMA queue so the x.T tiles (sync
        # queue) stream in parallel — the first projection only waits for
        # w chunk-group 0 + xt chunk-group 0 instead of ~7MB of constants.
        w_sb = const.tile([128, NCH, WCOLS], F16)
        for cg in range(4):
            nc.scalar.dma_start(
                out=w_sb[:, cg * 4:(cg + 1) * 4, :],
                in_=w_d[cg * 512:(cg + 1) * 512, :].rearrange(
                    "(c p) n -> p c n", p=128))
        mask_sb = const.tile([128, 128], F16)
        nc.scalar.dma_start(out=mask_sb[:], in_=mask_d[:])
        idlo = const.tile([128, 64], F16)
        nc.scalar.dma_start(out=idlo[:], in_=idlo_d[:])
        id32 = const.tile([128, 128], F32)
        nc.scalar.dma_start(out=id32[:], in_=id32_d[:])
        # rope tables in [d-row, s] layout, fp16; ~4KB/partition each
        cq = const.tile([128, S], F16)
        nc.scalar.dma_start(out=cq[:], in_=cq_d[:])
        sq = const.tile([128, S], F16)
        nc.scalar.dma_start(out=sq[:], in_=sq_d[:])
        ck = const.tile([128, S], F16)
        nc.scalar.dma_start(out=ck[:], in_=ck_d[:])
        sk = const.tile([128, S], F16)
        nc.scalar.dma_start(out=sk[:], in_=sk_d[:])

        # rows 0-63: K.T (RoPE'd); rows 64-127: duplicate copy so that the
        # scores matmul lhsT can match either base partition of the Q halves
        kt_res = resid.tile([128, B * S], F16)
        vp_res = resid.tile([128, B * NKV, 128], F16)  # [V|1|0pad] kv-tiles
        nc.vector.memset(vp_res[:], 0.0)
        nc.vector.memset(vp_res[:, :, 64:65], 1.0)

        for st in range(NST):
            b, sti = divmod(st, 4)
            ssl = slice(sti * ST, (sti + 1) * ST)  # within-batch s range

            # ---- x.T tile straight from HBM (host-transposed), split so the
            # first chunk-group's projections can start before the rest land
            xt = xt_p.tile([128, NCH, ST], F16, tag="xt")
            for cg in range(4):
                nc.sync.dma_start(
                    out=xt[:, cg * 4:(cg + 1) * 4, :],
                    in_=xt_d[cg * 512:(cg + 1) * 512,
                             st * ST:(st + 1) * ST].rearrange(
                                 "(c p) s -> p c s", p=128))

            # ---- W-stationary projections + RoPE in [d, s] layout ----
            # wb 0: [h0_e h0_o h1_e h1_o], wb 1: [h2_e h2_o h3_e h3_o],
            # wb 2: [k_e k_o | V(64 natural cols, transposed layout)]
            qta = qt_p.tile([128, ST], F16, tag="qta")
            qtb = qt_p.tile([128, ST], F16, tag="qtb")
            kvq = kvq_p.tile([128, ST], F16, tag="kvq")
            for wb in range(3):
                ppt = pr_ps.tile([128, ST], F32, tag="ppt")
                for c in range(NCH):
                    nc.tensor.matmul(
                        ppt[:], w_sb[:, c, wb * 128:(wb + 1) * 128],
                        xt[:, c, :], start=(c == 0), stop=(c == NCH - 1))
                # partner rows for the rotation (32-part cross-quadrant
                # copies), then qdst = ppt*C + partner*S_signed
                sh = ro_p.tile([128, ST], F32, tag="sh")
                nc.vector.tensor_copy(sh[0:32, :], ppt[32:64, :])
                nc.vector.tensor_copy(sh[32:64, :], ppt[0:32, :])
                if wb < 2:
                    nc.vector.tensor_copy(sh[64:96, :], ppt[96:128, :])
                    nc.vector.tensor_copy(sh[96:128, :], ppt[64:96, :])
                    qdst, tc_, ts_ = (qta if wb == 0 else qtb), cq, sq
                else:
                    # V rows pass through (ck=1, sk=0); keep sh initialized
                    nc.vector.tensor_copy(sh[64:128, :], ppt[64:128, :])
                    qdst, tc_, ts_ = kvq, ck, sk
                ts2 = ro_p.tile([128, ST], F32, tag="ts2")
                nc.vector.tensor_tensor(qdst[:], ppt[:], tc_[:, ssl], MUL)
                nc.vector.tensor_tensor(ts2[:], sh[:], ts_[:, ssl], MUL)
                nc.vector.tensor_tensor(qdst[:], qdst[:], ts2[:], ADD)
            # K.T rows into the resident (plus base-64 duplicate via DMA)
            nc.vector.tensor_copy(
                kt_res[0:64, st * ST:(st + 1) * ST], kvq[0:64, :])
            nc.sync.dma_start(
                out=kt_res[64:128, st * ST:(st + 1) * ST],
                in_=kt_res[0:64, st * ST:(st + 1) * ST])
            # V.T -> [kv, d] natural via small PE transposes
            for vc in range(4):
                tpv = tp_ps.tile([128, 64], F16, tag="tp")
                nc.tensor.transpose(
                    tpv[:], kvq[64:128, vc * 128:(vc + 1) * 128],
                    idlo[64:128, :])
                nc.vector.tensor_copy(
                    vp_res[:, b * NKV + sti * 4 + vc, 0:64], tpv[:])

            # ---- attention for the 4 heads of this q-tile ----
            js = [4 * sti] + list(range(4 * sti)) + \
                 [4 * sti + 1, 4 * sti + 2, 4 * sti + 3]
            for h in range(HPC):
                p0 = (h % 2) * 64
                qh = (qta if h < 2 else qtb)[p0:p0 + 64, :]
                cxt = cx_ps.tile([128, ST], F32, tag="cxt")
                for idx, j in enumerate(js):
                    off = 128 * j - 512 * sti
                    w0 = max(0, off)
                    sc = sc_ps.tile([128, ST], F32, tag="sc")
                    nc.tensor.matmul(
                        sc[:, w0:ST],
                        kt_res[p0:p0 + 64, b * S + j * 128:b * S + (j + 1) * 128],
                        qh[:, w0:ST], start=True, stop=True)
                    psb = p_p.tile([128, ST], F16, tag="psb")
                    nc.scalar.activation(
                        psb[:, w0:ST], sc[:, w0:ST],
                        mybir.ActivationFunctionType.Exp, scale=0.125)
                    if j >= 4 * sti:
                        # zero the upper-triangle of the diagonal block
                        nc.vector.tensor_tensor(
                            psb[:, off:off + 128], psb[:, off:off + 128],
                            mask_sb[:], MUL)
                    nc.tensor.matmul(
                        cxt[:, w0:ST], vp_res[:, b * NKV + j, :],
                        psb[:, w0:ST],
                        start=(idx == 0), stop=(idx == len(js) - 1))
                cxs = cx_p.tile([65, ST], F32, tag="cxs")
                nc.vector.tensor_copy(cxs[:], cxt[0:65, :])
                # fi shares the cx_ps buffers (same tag/shape as cxt)
                fi = cx_ps.tile([128, ST], F32, tag="cxt")
                for qq in range(4):
                    nc.tensor.transpose(
                        fi[:, qq * 128:qq * 128 + 66],
                        cxs[:, qq * 128:(qq + 1) * 128],
                        id32[0:65, 0:66])
                rv = rv_p.tile([128, 4], F32, tag="rv")
                nc.vector.reciprocal(rv[:], fi[:, 64:ST:128])
                ob = o_p.tile([128, 4, 64], F32, tag="ob")
                for qq in range(4):
                    nc.vector.tensor_scalar_mul(
                        ob[:, qq, :], fi[:, qq * 128:qq * 128 + 64],
                        rv[:, qq:qq + 1])
                nc.sync.dma_start(
                    out=out_d[st * ST:(st + 1) * ST,
                              h * 64:(h + 1) * 64].rearrange(
                                  "(q p) d -> p q d", p=128),
                    in_=ob[:])
    return nc


_NC_CACHE = None


def _host_consts():
    i = np.arange(0, D, 2, dtype=np.float64) / D          # 32 pair exponents
    freqs = 1.0 / (10000.0 ** i)                           # (32,)
    ang = np.arange(S, dtype=np.float64)[:, None] * freqs[None, :]  # (S, 32)
    cos32 = np.cos(ang).astype(np.float32).T               # (32, S)
    sin32 = np.sin(ang).astype(np.float32).T
    ones = np.ones((64, S), np.float32)
    zeros = np.zeros((64, S), np.float32)
    cq = np.vstack([cos32, cos32, cos32, cos32]).astype(np.float16)
    sq = np.vstack([-sin32, sin32, -sin32, sin32]).astype(np.float16)
    ck = np.vstack([cos32, cos32, ones]).astype(np.float16)
    sk = np.vstack([-sin32, sin32, zeros]).astype(np.float16)
    kv, qq = np.meshgrid(np.arange(128), np.arange(128), indexing="ij")
    mask01 = (kv <= qq).astype(np.float16)                 # 1 = allowed
    idlo = np.zeros((128, 64), np.float16)
    idlo[64:128] = np.eye(64, dtype=np.float16)
    ident32 = np.eye(128, dtype=np.float32)
    return cq, sq, ck, sk, mask01, idlo, ident32


def _deint(w):
    # de-interleave rope pairs per 64-col head: [evens | odds]
    return np.hstack([w[:, 0::2], w[:, 1::2]])


def _in_maps(x, Wq, Wk, Wv):
    x = np.asarray(x, dtype=np.float32).reshape(B * S, DIN)
    xt = np.ascontiguousarray(x.T).astype(np.float16)      # [DIN, B*S]
    Wq = np.asarray(Wq, dtype=np.float32)
    Wk = np.asarray(Wk, dtype=np.float32)
    Wv = np.asarray(Wv, dtype=np.float32)
    cq, sq, ck, sk, mask01, idlo, ident32 = _host_consts()

    in_maps = []
    for k in range(NCORES):
        cols = []
        for h in range(4):
            cols.append(_deint(Wq[:, (4 * k + h) * 64:(4 * k + h + 1) * 64]))
        cols.append(_deint(Wk[:, k * 64:(k + 1) * 64]))
        cols.append(Wv[:, k * 64:(k + 1) * 64])
        w_all = np.hstack(cols).astype(np.float16)
        in_maps.append({
            "xt": xt, "w": np.ascontiguousarray(w_all),
            "cq": cq, "sq": sq, "ck": ck, "sk": sk,
            "mask": mask01, "idlo": idlo, "id32": ident32,
        })
    return in_maps


def _run(in_maps, **kwargs):
    global _NC_CACHE
    if _NC_CACHE is None:
        _NC_CACHE = build_bass()
        _NC_CACHE.finalize()
    return run_bass_kernel_spmd(_NC_CACHE, in_maps, list(range(NCORES)),
                                **kwargs)


def kernel(x, Wq, Wk, Wv):
    res = _run(_in_maps(x, Wq, Wk, Wv))
    out = np.concatenate([res.results[k]["out"] for k in range(NCORES)], axis=1)
    return out.reshape(B, S, 32 * D)
